# revision 37
# baseline (speedup 1.0000x reference)
"""Trainium2 Bass kernel for causal cross-attention (B=2,S=2048,D=1024,H=16).

Sharding: 8 cores = 2 (batch) x 4 (head groups of 4 heads). Each core computes
its 4 heads' attention + its slice of the output projection; host sums the 4
fp16 partial outputs per batch element (fp32 accumulate) and adds the biases.

All matmul operands are fp16 (PSUM stays fp32): halves DMA traffic, dodges the
f32r short-row penalty, and doubles DVE throughput on SBUF elementwise ops.

The attention phase is Activation-engine bound (exp is ~58us of pure ACT work
that nothing else can absorb), so the kernel is built as a single software
pipeline: the q/k/v projection is cut into small single-psum-slot passes
(q chunks of 512 s, k+v eighths of 256 s) that are queued as "filler" units
and emitted one per attention inner-loop iteration. Attention on chunk j only
needs the first 2(j+1) eighths, so exp starts ~15us into the kernel and the
PE-side projection work hides under the ACT-bound attention stream.

Device-side layouts (per core):
  qT[j], kT[c]  [128, 2, 512] fp16; partition = (par=h%2)*64 + hd, free dim
                o = h//2 (=hp), s-chunk of 512.
  vaug[sv]      [128, 4, 65] fp16: v rows (keys on partitions) + ones column
                per head so AV also produces the softmax denominator.
  AV psum       [128 q, 4 st, 65] fp32 per head: q on PARTITIONS (full 128)
                so AV matmuls cost 65 rows instead of 512, and the softmax
                denominator lands as a per-partition scalar (cheap reciprocal
                + normalize, no broadcast matmul).
  o_sb          [128 q, 4 st, 2 par, 64] fp16 normalized attention out.
  oTT[j]        [128 d, 2 hp, 4 st, 128 q] fp16 via one DMA XBAR transpose
                per (j, hp); d = par*64 + hd stacks the two heads of an hp so
                the output projection contracts over the full 128 partitions.
Causal masking: k-tile/q-subtile pairs are skipped entirely at 128-granularity
(exact there); only the diagonal 128x128 block needs a multiplicative triangle
mask (the SAME triangle for every diagonal block). Softmax skips the
max-subtract (scores ~ N(0,1), exp can't overflow fp16).
DMA queues: x/y/out on sync (SP), weights on scalar (ACT, idle pre-exp),
transposes on vector (DVE, right behind the norm ops they depend on).
"""

import sys

sys.path.insert(0, "/opt/trn_rl_repo")

from collections import deque
from contextlib import ExitStack

import numpy as np

import concourse.bass as bass
import concourse.tile as tile
from concourse import bacc
from concourse import mybir
from concourse.bass_utils import run_bass_kernel_spmd

B, S, D, H = 2, 2048, 1024, 16
HD = 64                      # head dim
SCALE = HD ** -0.5
HG = 4                       # heads per core
DL = HG * HD                 # 256 local projection dims per core
P = 128
NJ = S // 512                # 4 q chunks
KD = D // P                  # 8 din tiles
VA = HD + 1                  # 65: v columns per head incl. ones column

F32 = mybir.dt.float32
F16 = mybir.dt.float16
EXP = mybir.ActivationFunctionType.Exp

_PROG = None


def _build_program(iters=None):
    nc = bacc.Bacc()
    p_xT = nc.declare_dram_parameter("xT", [D, S], F16, isOutput=False)
    p_yT = nc.declare_dram_parameter("yT", [D, S], F16, isOutput=False)
    p_wq = nc.declare_dram_parameter("wq", [D, DL], F16, isOutput=False)
    p_wk = nc.declare_dram_parameter("wk", [D, DL], F16, isOutput=False)
    p_wv = nc.declare_dram_parameter("wv", [D, DL], F16, isOutput=False)
    p_wo = nc.declare_dram_parameter("wo", [DL, D], F16, isOutput=False)
    p_bq = nc.declare_dram_parameter("bq", [DL], F32, isOutput=False)
    p_bk = nc.declare_dram_parameter("bk", [DL], F32, isOutput=False)
    p_tri = nc.declare_dram_parameter("tri", [P, 1, P], F16, isOutput=False)
    p_ones = nc.declare_dram_parameter("ones_c", [P, HG], F16, isOutput=False)
    p_out = nc.declare_dram_parameter("out", [S, D], F16, isOutput=True)

    with tile.TileContext(nc) as tc, ExitStack() as ctx:
        singles = ctx.enter_context(tc.tile_pool(name="singles", bufs=1))
        # PSUM: spool = 2 x 2-bank scores ping/pong, fpool = 1 x 2-bank slot
        # rotating between projection passes and out-proj groups (decoupled
        # from the scores pipeline), pav = 2 x 1-bank AV accumulators
        spool = ctx.enter_context(
            tc.tile_pool(name="spool", bufs=2, space="PSUM"))
        fpool = ctx.enter_context(
            tc.tile_pool(name="fpool", bufs=1, space="PSUM"))
        pav = ctx.enter_context(tc.tile_pool(name="pav", bufs=2, space="PSUM"))
        epool = ctx.enter_context(tc.tile_pool(name="exp", bufs=6))
        opool = ctx.enter_context(tc.tile_pool(name="o", bufs=8))
        rpool = ctx.enter_context(tc.tile_pool(name="recip", bufs=3))
        outp = ctx.enter_context(tc.tile_pool(name="outp", bufs=4))

        def emit_body():
            qT = [singles.tile([P, 2, 512], F16, tag=f"qT{j}", name=f"qT_{j}")
                  for j in range(NJ)]
            kT = [singles.tile([P, 2, 512], F16, tag=f"kT{c}", name=f"kT_{c}")
                  for c in range(4)]
            vaug = [singles.tile([P, HG, VA], F16, tag=f"va{sv}",
                                 name=f"va_{sv}") for sv in range(16)]
            xF = singles.tile([P, KD, S], F16, tag="xF")
            yF = singles.tile([P, KD, S], F16, tag="yF")
            oTT = [singles.tile([P, 2, 4, P], F16, tag=f"oTT{j}",
                                name=f"oTT_{j}") for j in range(NJ)]
            wq_sb = singles.tile([P, KD, DL], F16, tag="wq")
            wk_sb = singles.tile([P, KD, DL], F16, tag="wk")
            wv_sb = singles.tile([P, KD, DL], F16, tag="wv")
            wo2_sb = singles.tile([P, 2, D], F16, tag="wo2")
            bq_sb = singles.tile([P, 2, 1], F32, tag="bq")
            bk_sb = singles.tile([P, 2, 1], F32, tag="bk")
            tri = singles.tile([P, 1, P], F16, tag="tri")
            ones = singles.tile([P, HG], F16, tag="ones")
            warm = singles.tile([P, 1], F16, tag="warm")

            qps = {}

            def emit_qquarter(j, part):
                if part == 0:
                    qps[j] = fpool.tile([P, 2, 512], F32, tag="fp",
                                        name=f"qp_{j}")
                qp = qps[j]
                for i in range(4 * part, 4 * part + 4):
                    for d in range(2):
                        nc.tensor.matmul(
                            qp[:, d, :],
                            lhsT=wq_sb[:, i, P * d:P * d + P],
                            rhs=xF[:, i, 512 * j:512 * j + 512],
                            start=(i == 0), stop=(i == KD - 1))
                if part == 1:
                    nc.vector.tensor_add(
                        qT[j], qp, bq_sb.to_broadcast((P, 2, 512)))

            t8s = {}

            def emit_eighth(e, part):
                c, eh = divmod(e, 2)
                so = 256 * e
                if part == 0:
                    t8s[e] = fpool.tile([P, 2, 512], F32, tag="fp",
                                        name=f"t8_{e}")
                t8 = t8s[e]
                for i in range(4 * part, 4 * part + 4):
                    for d in range(2):
                        nc.tensor.matmul(
                            t8[:, 0, 256 * d:256 * d + 256],
                            lhsT=wk_sb[:, i, P * d:P * d + P],
                            rhs=yF[:, i, so:so + 256],
                            start=(i == 0 and d == 0), stop=(i == KD - 1),
                            skip_group_check=(d == 1))
                    for r in range(2):
                        nc.tensor.matmul(
                            t8[:, 1, 256 * r:256 * r + 256],
                            lhsT=yF[:, i, so + P * r:so + P * r + P],
                            rhs=wv_sb[:, i, :],
                            start=(i == 0 and r == 0), stop=(i == KD - 1),
                            skip_group_check=(r == 1))
                if part == 1:
                    nc.vector.tensor_add(
                        kT[c][:, :, 256 * eh:256 * eh + 256],
                        t8[:, 0, :].rearrange("p (d s) -> p d s", s=256),
                        bk_sb.to_broadcast((P, 2, 256)))
                    for r in range(2):
                        sv = 2 * e + r
                        v_src = t8[:, 1, 256 * r:256 * r + 256].rearrange(
                            "p (h c) -> p h c", c=HD)
                        nc.vector.tensor_copy(
                            out=vaug[sv][:, :, 0:HD], in_=v_src)
                        nc.vector.tensor_copy(
                            out=vaug[sv][:, :, HD], in_=ones)

            def emit_transpose(j, hp, o_sb):
                nc.sync.dma_start(
                    oTT[j][:, hp, :, :], o_sb[:, :, :, :], transpose=True)

            def emit_outproj_group(j, tt, pool=None):
                t = 4 * j + tt
                o_pt = (pool or fpool).tile(
                    [P, 2, 512], F32, tag="fp" if pool is None else "sp",
                    name=f"opt_{t}")
                for nh in range(2):
                    for hp in range(2):
                        nc.tensor.matmul(
                            o_pt[:, nh, :],
                            lhsT=oTT[j][:, hp, tt, :],
                            rhs=wo2_sb[:, hp, 512 * nh:512 * nh + 512],
                            start=(hp == 0), stop=(hp == 1))
                o_fin = outp.tile([P, 2, 512], F16, tag="of", name=f"of_{t}")
                if pool is None and j <= 1:
                    # mid-phase: ACT has idle capacity; halving the drain
                    # shortens the DVE queue ahead of recip/norm
                    nc.scalar.copy(o_fin[:, 0, :], o_pt[:, 0, :])
                    nc.vector.tensor_copy(
                        out=o_fin[:, 1, :], in_=o_pt[:, 1, :])
                elif pool is None:
                    nc.vector.tensor_copy(out=o_fin, in_=o_pt)
                else:
                    # tail flush: ACT is idle once the exps are done; split
                    # the drain across both engines
                    nc.scalar.copy(o_fin[:, 0, :], o_pt[:, 0, :])
                    nc.vector.tensor_copy(
                        out=o_fin[:, 1, :], in_=o_pt[:, 1, :])
                nc.sync.dma_start(p_out[P * t:P * t + P, :], o_fin)

            # ---- prologue. Few, large DMAs: HWDGE issue overhead (~630ns
            # each) was the prologue bottleneck with per-tile streams. The
            # s<512 quarter of x and y (all attention chunk 0 needs) loads
            # first so exp starts ~12us in; the rest follows as one big DMA.
            x_r = p_xT.rearrange("(i p) s -> p i s", p=P)
            y_r = p_yT.rearrange("(i p) s -> p i s", p=P)
            wq_r = p_wq.rearrange("(o p) m -> p o m", p=P)
            # din-0 slivers first: the very first matmul is gated only by
            # these two small transfers, not the full weight/x loads
            nc.scalar.dma_start(wq_sb[:, 0, :], wq_r[:, 0, :])
            nc.sync.dma_start(xF[:, 0, 0:512], x_r[:, 0, 0:512])
            nc.scalar.dma_start(wq_sb[:, 1:KD, :], wq_r[:, 1:KD, :])
            nc.sync.dma_start(xF[:, 1:KD, 0:512], x_r[:, 1:KD, 0:512])
            nc.scalar.dma_start(wk_sb, p_wk.rearrange("(o p) m -> p o m", p=P))
            nc.scalar.dma_start(wv_sb, p_wv.rearrange("(o p) m -> p o m", p=P))
            nc.sync.dma_start(yF[:, :, 0:512], y_r[:, :, 0:512])
            nc.scalar.dma_start(bq_sb, p_bq.rearrange("(o p) -> p o", p=P))
            nc.scalar.dma_start(bk_sb, p_bk.rearrange("(o p) -> p o", p=P))
            nc.scalar.dma_start(ones, p_ones[:, :])
            nc.sync.dma_start(xF[:, :, 512:1024], x_r[:, :, 512:1024])
            nc.sync.dma_start(yF[:, :, 512:1024], y_r[:, :, 512:1024])
            nc.sync.dma_start(xF[:, :, 1024:2048], x_r[:, :, 1024:2048])
            nc.sync.dma_start(yF[:, :, 1024:2048], y_r[:, :, 1024:2048])
            emit_qquarter(0, 0)
            emit_qquarter(0, 1)
            # absorb the activation-table load before the first real exp
            nc.scalar.activation(warm, ones[:, 0:1], EXP)
            nc.scalar.dma_start(wo2_sb, p_wo.rearrange(
                "(hp par hd) n -> (par hd) hp n", hp=2, par=2, hd=HD))
            nc.scalar.dma_start(tri, p_tri[:, :, :])
            for e in (0, 1):
                emit_eighth(e, 0)
                emit_eighth(e, 1)

            # filler FIFO: remaining projection passes in dependency order,
            # out-proj groups appended as chunks finish. One unit is popped
            # per attention inner-loop iteration; deadlines flush before the
            # chunk that needs them.
            filler = deque()
            n_static = 0

            def add_static(fn, *args):
                nonlocal n_static
                n_static += 1
                filler.append(("s", fn, args))

            for grp in ((1,), (2, 3), (2,), (4, 5), (3,), (6, 7)):
                if len(grp) == 1:
                    for part in range(2):
                        add_static(emit_qquarter, grp[0], part)
                else:
                    for e in grp:
                        for part in range(2):
                            add_static(emit_eighth, e, part)

            static_done = 0

            def pop_filler():
                nonlocal static_done
                if not filler:
                    return
                kind, fn, args = filler.popleft()
                if kind == "s":
                    static_done += 1
                fn(*args)

            def flush_static(n):
                while static_done < n:
                    pop_filler()

            # chunk j needs qT[j] (quarter j) and kT[0..j] (eighths 0..2j+1):
            # statics are queued as [Q1, E2, E3, Q2, E4, E5, Q3, E6, E7] x2
            deadlines = {1: 6, 2: 12, 3: 18}

            # ---- attention
            for j in range(NJ):
                if j in deadlines:
                    flush_static(deadlines[j])
                nk = 4 * j + 4
                for hp in range(2):
                    av = [pav.tile([P, HG, VA], F32, tag="av",
                                   name=f"av_{j}_{hp}_{par}")
                          for par in range(2)]

                    def emit_av(i, e_sb, av=av, hp=hp, j=j):
                        m = i - 4 * j
                        for par in range(2):
                            h = 2 * hp + par
                            for st in range(max(0, m), 4):
                                nc.tensor.matmul(
                                    av[par][:, st, :],
                                    lhsT=e_sb[:, par, P * st:P * st + P],
                                    rhs=vaug[i][:, h, :],
                                    start=(i == 0 and st == 0),
                                    stop=(i == 4 * j + st),
                                    skip_group_check=not (i == 0 and st == 0))

                    pend = None
                    for i in range(nk):
                        m = i - 4 * j
                        lo = max(0, P * m)
                        c, ir = divmod(i, 4)
                        sp = spool.tile([P, 2, 512], F32, tag="sp")
                        for par in range(2):
                            base = HD * par
                            nc.tensor.matmul(
                                sp[:, par, lo:],
                                lhsT=kT[c][base:base + HD, hp,
                                           P * ir:P * ir + P],
                                rhs=qT[j][base:base + HD, hp, lo:],
                                start=True, stop=True)
                        e_sb = epool.tile([P, 2, 512], F16, tag="e")
                        nc.scalar.activation(
                            e_sb[:, :, lo:], sp[:, :, lo:], EXP, scale=SCALE)
                        if m >= 0:
                            nc.vector.tensor_mul(
                                e_sb[:, :, lo:lo + P],
                                e_sb[:, :, lo:lo + P],
                                tri.to_broadcast((P, 2, P)))
                        if pend is not None:
                            emit_av(*pend)
                            # later chunks have more attention work per
                            # iteration; thin the filler stream so some
                            # remains to plug late-chunk pipeline bubbles
                            if j < 2 or i % 2 == 1:
                                pop_filler()
                        pend = (i, e_sb)
                    # fill the PE queue while the last exp drains: the final
                    # AV matmuls otherwise stall the PE ~1us per hp
                    pop_filler()
                    pop_filler()
                    emit_av(*pend)

                    o_sb = opool.tile([P, 4, 2, HD], F16, tag="o",
                                      name=f"osb_{j}_{hp}")
                    rcps = []
                    for par in range(2):
                        rcp = rpool.tile([P, HG, 1], F32, tag="r")
                        nc.vector.reciprocal(rcp, av[par][:, :, HD:VA])
                        rcps.append(rcp)
                    if j == NJ - 1 and hp == 1:
                        # last stage: st-granular so each q-subtile's
                        # transpose + out-proj can start while later subtiles
                        # still normalize (shortens the serial tail)
                        for st in range(4):
                            for par in range(2):
                                nc.vector.tensor_mul(
                                    o_sb[:, st, par, :],
                                    av[par][:, st, 0:HD],
                                    rcps[par][:, st, :].to_broadcast((P, HD)))
                            nc.sync.dma_start(
                                oTT[j][:, hp, st, :], o_sb[:, st, :, :],
                                transpose=True)
                    else:
                        for par in range(2):
                            nc.vector.tensor_mul(
                                o_sb[:, :, par, :], av[par][:, :, 0:HD],
                                rcps[par].to_broadcast((P, HG, HD)))
                        # the transpose is deferred into the filler stream so
                        # the SP queue never parks on its norm dependency
                        filler.append(("d", emit_transpose, (j, hp, o_sb)))
                filler.extend(
                    ("d", emit_outproj_group, (j, tt)) for tt in range(4))
            # tail flush: scores/AV psum is free now, so let out-proj groups
            # pipeline through the scores pool instead of the single fpool slot
            while filler:
                kind, fn, args = filler.popleft()
                if fn is emit_outproj_group:
                    fn(*args, pool=spool)
                else:
                    fn(*args)

        if iters is None:
            emit_body()
        else:
            with tc.For_i(0, iters, 1):
                emit_body()
    nc.compile()
    return nc


def _get_program():
    global _PROG
    if _PROG is None:
        _PROG = _build_program()
    return _PROG


def run(inputs, trace=False):
    x = np.asarray(inputs["x"], np.float32)
    y = np.asarray(inputs["y"], np.float32)
    Wq = np.asarray(inputs["Wq"], np.float32)
    Wk = np.asarray(inputs["Wk"], np.float32)
    Wv = np.asarray(inputs["Wv"], np.float32)
    Wo = np.asarray(inputs["Wo"], np.float32)
    bq = np.asarray(inputs["bq"], np.float32)
    bk = np.asarray(inputs["bk"], np.float32)
    bv = np.asarray(inputs["bv"], np.float32)
    bo = np.asarray(inputs["bo"], np.float32)

    nc = _get_program()
    tri = (np.arange(P)[None, :] >= np.arange(P)[:, None]).astype(
        np.float16).reshape(P, 1, P)
    ones_c = np.ones((P, HG), np.float16)
    xTs = [np.ascontiguousarray(x[b].T.astype(np.float16)) for b in range(B)]
    yTs = [np.ascontiguousarray(y[b].T.astype(np.float16)) for b in range(B)]

    in_maps = []
    for c in range(8):
        b, hg = divmod(c, HG)
        sl = slice(DL * hg, DL * hg + DL)
        in_maps.append({
            "xT": xTs[b],
            "yT": yTs[b],
            "wq": np.ascontiguousarray(Wq[:, sl].astype(np.float16)),
            "wk": np.ascontiguousarray(Wk[:, sl].astype(np.float16)),
            "wv": np.ascontiguousarray(Wv[:, sl].astype(np.float16)),
            "wo": np.ascontiguousarray(Wo[sl, :].astype(np.float16)),
            "bq": np.ascontiguousarray(bq[sl]),
            "bk": np.ascontiguousarray(bk[sl]),
            "tri": tri,
            "ones_c": ones_c,
        })

    res = run_bass_kernel_spmd(nc, in_maps, list(range(8)), trace=trace)
    extra = bv @ Wo + bo
    out = np.empty((B, S, D), np.float32)
    for b in range(B):
        acc = res.results[HG * b]["out"].astype(np.float32)
        for hg in range(1, HG):
            acc = acc + res.results[HG * b + hg]["out"].astype(np.float32)
        out[b] = acc + extra
    return out, res


def kernel(**inputs):
    out, _ = run(inputs, trace=False)
    return out


# revision 38
# speedup vs baseline: 1.0144x; 1.0144x over previous
"""Trainium2 Bass kernel for causal cross-attention (B=2,S=2048,D=1024,H=16).

Sharding: 8 cores = 2 (batch) x 4 (head groups of 4 heads). Each core computes
its 4 heads' attention + its slice of the output projection; host sums the 4
fp16 partial outputs per batch element (fp32 accumulate) and adds the biases.

All matmul operands are fp16 (PSUM stays fp32): halves DMA traffic, dodges the
f32r short-row penalty, and doubles DVE throughput on SBUF elementwise ops.

The attention phase is Activation-engine bound (exp is ~58us of pure ACT work
that nothing else can absorb), so the kernel is built as a single software
pipeline: the q/k/v projection is cut into small single-psum-slot passes
(q chunks of 512 s, k+v eighths of 256 s) that are queued as "filler" units
and emitted one per attention inner-loop iteration. Attention on chunk j only
needs the first 2(j+1) eighths, so exp starts ~15us into the kernel and the
PE-side projection work hides under the ACT-bound attention stream.

Device-side layouts (per core):
  qT[j], kT[c]  [128, 2, 512] fp16; partition = (par=h%2)*64 + hd, free dim
                o = h//2 (=hp), s-chunk of 512.
  vaug[sv]      [128, 4, 65] fp16: v rows (keys on partitions) + ones column
                per head so AV also produces the softmax denominator.
  AV psum       [128 q, 4 st, 65] fp32 per head: q on PARTITIONS (full 128)
                so AV matmuls cost 65 rows instead of 512, and the softmax
                denominator lands as a per-partition scalar (cheap reciprocal
                + normalize, no broadcast matmul).
  o_sb          [128 q, 4 st, 2 par, 64] fp16 normalized attention out.
  oTT[j]        [128 d, 2 hp, 4 st, 128 q] fp16 via one DMA XBAR transpose
                per (j, hp); d = par*64 + hd stacks the two heads of an hp so
                the output projection contracts over the full 128 partitions.
Causal masking: k-tile/q-subtile pairs are skipped entirely at 128-granularity
(exact there); only the diagonal 128x128 block needs a multiplicative triangle
mask (the SAME triangle for every diagonal block). Softmax skips the
max-subtract (scores ~ N(0,1), exp can't overflow fp16).
DMA queues: x/y/out on sync (SP), weights on scalar (ACT, idle pre-exp),
transposes on vector (DVE, right behind the norm ops they depend on).
"""

import sys

sys.path.insert(0, "/opt/trn_rl_repo")

from collections import deque
from contextlib import ExitStack

import numpy as np

import concourse.bass as bass
import concourse.tile as tile
from concourse import bacc
from concourse import mybir
from concourse.bass_utils import run_bass_kernel_spmd

B, S, D, H = 2, 2048, 1024, 16
HD = 64                      # head dim
SCALE = HD ** -0.5
HG = 4                       # heads per core
DL = HG * HD                 # 256 local projection dims per core
P = 128
NJ = S // 512                # 4 q chunks
KD = D // P                  # 8 din tiles
VA = HD + 1                  # 65: v columns per head incl. ones column

F32 = mybir.dt.float32
F16 = mybir.dt.float16
EXP = mybir.ActivationFunctionType.Exp

_PROG = None


def _build_program(iters=None):
    nc = bacc.Bacc()
    p_xT = nc.declare_dram_parameter("xT", [D, S], F16, isOutput=False)
    p_yT = nc.declare_dram_parameter("yT", [D, S], F16, isOutput=False)
    p_wq = nc.declare_dram_parameter("wq", [D, DL], F16, isOutput=False)
    p_wk = nc.declare_dram_parameter("wk", [D, DL], F16, isOutput=False)
    p_wv = nc.declare_dram_parameter("wv", [D, DL], F16, isOutput=False)
    p_wo = nc.declare_dram_parameter("wo", [DL, D], F16, isOutput=False)
    p_bq = nc.declare_dram_parameter("bq", [DL], F32, isOutput=False)
    p_bk = nc.declare_dram_parameter("bk", [DL], F32, isOutput=False)
    p_tri = nc.declare_dram_parameter("tri", [P, 1, P], F16, isOutput=False)
    p_ones = nc.declare_dram_parameter("ones_c", [P, HG], F16, isOutput=False)
    p_out = nc.declare_dram_parameter("out", [S, D], F16, isOutput=True)

    with tile.TileContext(nc) as tc, ExitStack() as ctx:
        singles = ctx.enter_context(tc.tile_pool(name="singles", bufs=1))
        # PSUM: spool = 2 x 2-bank scores ping/pong, fpool = 1 x 2-bank slot
        # rotating between projection passes and out-proj groups (decoupled
        # from the scores pipeline), pav = 2 x 1-bank AV accumulators
        spool = ctx.enter_context(
            tc.tile_pool(name="spool", bufs=2, space="PSUM"))
        fpool = ctx.enter_context(
            tc.tile_pool(name="fpool", bufs=1, space="PSUM"))
        pav = ctx.enter_context(tc.tile_pool(name="pav", bufs=2, space="PSUM"))
        epool = ctx.enter_context(tc.tile_pool(name="exp", bufs=6))
        opool = ctx.enter_context(tc.tile_pool(name="o", bufs=8))
        rpool = ctx.enter_context(tc.tile_pool(name="recip", bufs=3))
        outp = ctx.enter_context(tc.tile_pool(name="outp", bufs=4))

        def emit_body():
            qT = [singles.tile([P, 2, 512], F16, tag=f"qT{j}", name=f"qT_{j}")
                  for j in range(NJ)]
            kT = [singles.tile([P, 2, 512], F16, tag=f"kT{c}", name=f"kT_{c}")
                  for c in range(4)]
            vaug = [singles.tile([P, HG, VA], F16, tag=f"va{sv}",
                                 name=f"va_{sv}") for sv in range(16)]
            xF = singles.tile([P, KD, S], F16, tag="xF")
            yF = singles.tile([P, KD, S], F16, tag="yF")
            oTT = [singles.tile([P, 2, 4, P], F16, tag=f"oTT{j}",
                                name=f"oTT_{j}") for j in range(NJ)]
            wq_sb = singles.tile([P, KD, DL], F16, tag="wq")
            wk_sb = singles.tile([P, KD, DL], F16, tag="wk")
            wv_sb = singles.tile([P, KD, DL], F16, tag="wv")
            wo2_sb = singles.tile([P, 2, D], F16, tag="wo2")
            bq_sb = singles.tile([P, 2, 1], F32, tag="bq")
            bk_sb = singles.tile([P, 2, 1], F32, tag="bk")
            tri = singles.tile([P, 1, P], F16, tag="tri")
            ones = singles.tile([P, HG], F16, tag="ones")
            warm = singles.tile([P, 1], F16, tag="warm")

            qps = {}

            def emit_qquarter(j, part):
                if part == 0:
                    qps[j] = fpool.tile([P, 2, 512], F32, tag="fp",
                                        name=f"qp_{j}")
                qp = qps[j]
                for i in range(4 * part, 4 * part + 4):
                    for d in range(2):
                        nc.tensor.matmul(
                            qp[:, d, :],
                            lhsT=wq_sb[:, i, P * d:P * d + P],
                            rhs=xF[:, i, 512 * j:512 * j + 512],
                            start=(i == 0), stop=(i == KD - 1))
                if part == 1:
                    nc.vector.tensor_add(
                        qT[j], qp, bq_sb.to_broadcast((P, 2, 512)))

            t8s = {}

            def emit_eighth(e, part):
                c, eh = divmod(e, 2)
                so = 256 * e
                if part == 0:
                    t8s[e] = fpool.tile([P, 2, 512], F32, tag="fp",
                                        name=f"t8_{e}")
                t8 = t8s[e]
                for i in range(4 * part, 4 * part + 4):
                    for d in range(2):
                        nc.tensor.matmul(
                            t8[:, 0, 256 * d:256 * d + 256],
                            lhsT=wk_sb[:, i, P * d:P * d + P],
                            rhs=yF[:, i, so:so + 256],
                            start=(i == 0 and d == 0), stop=(i == KD - 1),
                            skip_group_check=(d == 1))
                    for r in range(2):
                        nc.tensor.matmul(
                            t8[:, 1, 256 * r:256 * r + 256],
                            lhsT=yF[:, i, so + P * r:so + P * r + P],
                            rhs=wv_sb[:, i, :],
                            start=(i == 0 and r == 0), stop=(i == KD - 1),
                            skip_group_check=(r == 1))
                if part == 1:
                    nc.vector.tensor_add(
                        kT[c][:, :, 256 * eh:256 * eh + 256],
                        t8[:, 0, :].rearrange("p (d s) -> p d s", s=256),
                        bk_sb.to_broadcast((P, 2, 256)))
                    for r in range(2):
                        sv = 2 * e + r
                        v_src = t8[:, 1, 256 * r:256 * r + 256].rearrange(
                            "p (h c) -> p h c", c=HD)
                        nc.vector.tensor_copy(
                            out=vaug[sv][:, :, 0:HD], in_=v_src)
                        nc.vector.tensor_copy(
                            out=vaug[sv][:, :, HD], in_=ones)

            def emit_transpose(j, hp, o_sb):
                nc.sync.dma_start(
                    oTT[j][:, hp, :, :], o_sb[:, :, :, :], transpose=True)

            def emit_outproj_group(j, tt, pool=None):
                t = 4 * j + tt
                o_pt = (pool or fpool).tile(
                    [P, 2, 512], F32, tag="fp" if pool is None else "sp",
                    name=f"opt_{t}")
                for nh in range(2):
                    for hp in range(2):
                        nc.tensor.matmul(
                            o_pt[:, nh, :],
                            lhsT=oTT[j][:, hp, tt, :],
                            rhs=wo2_sb[:, hp, 512 * nh:512 * nh + 512],
                            start=(hp == 0), stop=(hp == 1))
                o_fin = outp.tile([P, 2, 512], F16, tag="of", name=f"of_{t}")
                if pool is None:
                    nc.vector.tensor_copy(out=o_fin, in_=o_pt)
                else:
                    # tail flush: ACT is idle once the exps are done; split
                    # the drain across both engines
                    nc.scalar.copy(o_fin[:, 0, :], o_pt[:, 0, :])
                    nc.vector.tensor_copy(
                        out=o_fin[:, 1, :], in_=o_pt[:, 1, :])
                nc.sync.dma_start(p_out[P * t:P * t + P, :], o_fin)

            # ---- prologue. Few, large DMAs: HWDGE issue overhead (~630ns
            # each) was the prologue bottleneck with per-tile streams. The
            # s<512 quarter of x and y (all attention chunk 0 needs) loads
            # first so exp starts ~12us in; the rest follows as one big DMA.
            x_r = p_xT.rearrange("(i p) s -> p i s", p=P)
            y_r = p_yT.rearrange("(i p) s -> p i s", p=P)
            wq_r = p_wq.rearrange("(o p) m -> p o m", p=P)
            # din-0 slivers first: the very first matmul is gated only by
            # these two small transfers, not the full weight/x loads
            nc.scalar.dma_start(wq_sb[:, 0, :], wq_r[:, 0, :])
            nc.sync.dma_start(xF[:, 0, 0:512], x_r[:, 0, 0:512])
            nc.scalar.dma_start(wq_sb[:, 1:KD, :], wq_r[:, 1:KD, :])
            nc.sync.dma_start(xF[:, 1:KD, 0:512], x_r[:, 1:KD, 0:512])
            nc.scalar.dma_start(wk_sb, p_wk.rearrange("(o p) m -> p o m", p=P))
            nc.scalar.dma_start(wv_sb, p_wv.rearrange("(o p) m -> p o m", p=P))
            nc.sync.dma_start(yF[:, :, 0:512], y_r[:, :, 0:512])
            nc.scalar.dma_start(bq_sb, p_bq.rearrange("(o p) -> p o", p=P))
            nc.scalar.dma_start(bk_sb, p_bk.rearrange("(o p) -> p o", p=P))
            nc.scalar.dma_start(ones, p_ones[:, :])
            nc.sync.dma_start(xF[:, :, 512:1024], x_r[:, :, 512:1024])
            nc.sync.dma_start(yF[:, :, 512:1024], y_r[:, :, 512:1024])
            nc.sync.dma_start(xF[:, :, 1024:2048], x_r[:, :, 1024:2048])
            nc.sync.dma_start(yF[:, :, 1024:2048], y_r[:, :, 1024:2048])
            emit_qquarter(0, 0)
            emit_qquarter(0, 1)
            # absorb the activation-table load before the first real exp
            nc.scalar.activation(warm, ones[:, 0:1], EXP)
            nc.scalar.dma_start(wo2_sb, p_wo.rearrange(
                "(hp par hd) n -> (par hd) hp n", hp=2, par=2, hd=HD))
            nc.scalar.dma_start(tri, p_tri[:, :, :])
            for e in (0, 1):
                emit_eighth(e, 0)
                emit_eighth(e, 1)

            # filler FIFO: remaining projection passes in dependency order,
            # out-proj groups appended as chunks finish. One unit is popped
            # per attention inner-loop iteration; deadlines flush before the
            # chunk that needs them.
            filler = deque()
            n_static = 0

            def add_static(fn, *args):
                nonlocal n_static
                n_static += 1
                filler.append(("s", fn, args))

            for grp in ((1,), (2, 3), (2,), (4, 5), (3,), (6, 7)):
                if len(grp) == 1:
                    for part in range(2):
                        add_static(emit_qquarter, grp[0], part)
                else:
                    for e in grp:
                        for part in range(2):
                            add_static(emit_eighth, e, part)

            static_done = 0

            def pop_filler():
                nonlocal static_done
                if not filler:
                    return
                kind, fn, args = filler.popleft()
                if kind == "s":
                    static_done += 1
                fn(*args)

            def flush_static(n):
                while static_done < n:
                    pop_filler()

            # chunk j needs qT[j] (quarter j) and kT[0..j] (eighths 0..2j+1):
            # statics are queued as [Q1, E2, E3, Q2, E4, E5, Q3, E6, E7] x2
            deadlines = {1: 6, 2: 12, 3: 18}

            # ---- attention
            for j in range(NJ):
                if j in deadlines:
                    flush_static(deadlines[j])
                nk = 4 * j + 4
                for hp in range(2):
                    av = [pav.tile([P, HG, VA], F32, tag="av",
                                   name=f"av_{j}_{hp}_{par}")
                          for par in range(2)]

                    def emit_av(i, e_sb, av=av, hp=hp, j=j):
                        m = i - 4 * j
                        for par in range(2):
                            h = 2 * hp + par
                            for st in range(max(0, m), 4):
                                nc.tensor.matmul(
                                    av[par][:, st, :],
                                    lhsT=e_sb[:, par, P * st:P * st + P],
                                    rhs=vaug[i][:, h, :],
                                    start=(i == 0 and st == 0),
                                    stop=(i == 4 * j + st),
                                    skip_group_check=not (i == 0 and st == 0))

                    pend = None
                    for i in range(nk):
                        m = i - 4 * j
                        lo = max(0, P * m)
                        c, ir = divmod(i, 4)
                        sp = spool.tile([P, 2, 512], F32, tag="sp")
                        for par in range(2):
                            base = HD * par
                            nc.tensor.matmul(
                                sp[:, par, lo:],
                                lhsT=kT[c][base:base + HD, hp,
                                           P * ir:P * ir + P],
                                rhs=qT[j][base:base + HD, hp, lo:],
                                start=True, stop=True)
                        e_sb = epool.tile([P, 2, 512], F16, tag="e")
                        nc.scalar.activation(
                            e_sb[:, :, lo:], sp[:, :, lo:], EXP, scale=SCALE)
                        if m >= 0:
                            nc.vector.tensor_mul(
                                e_sb[:, :, lo:lo + P],
                                e_sb[:, :, lo:lo + P],
                                tri.to_broadcast((P, 2, P)))
                        if pend is not None:
                            emit_av(*pend)
                            # later chunks have more attention work per
                            # iteration; thin the filler stream so some
                            # remains to plug late-chunk pipeline bubbles
                            if j < 2 or i % 2 == 1:
                                pop_filler()
                        pend = (i, e_sb)
                    # fill the PE queue while the last exp drains: the final
                    # AV matmuls otherwise stall the PE ~1us per hp
                    pop_filler()
                    pop_filler()
                    emit_av(*pend)

                    o_sb = opool.tile([P, 4, 2, HD], F16, tag="o",
                                      name=f"osb_{j}_{hp}")
                    rcps = []
                    for par in range(2):
                        rcp = rpool.tile([P, HG, 1], F32, tag="r")
                        nc.vector.reciprocal(rcp, av[par][:, :, HD:VA])
                        rcps.append(rcp)
                    if j == NJ - 1 and hp == 1:
                        # last stage: st-granular so each q-subtile's
                        # transpose + out-proj can start while later subtiles
                        # still normalize (shortens the serial tail)
                        for st in range(4):
                            for par in range(2):
                                nc.vector.tensor_mul(
                                    o_sb[:, st, par, :],
                                    av[par][:, st, 0:HD],
                                    rcps[par][:, st, :].to_broadcast((P, HD)))
                            nc.sync.dma_start(
                                oTT[j][:, hp, st, :], o_sb[:, st, :, :],
                                transpose=True)
                    else:
                        for par in range(2):
                            nc.vector.tensor_mul(
                                o_sb[:, :, par, :], av[par][:, :, 0:HD],
                                rcps[par].to_broadcast((P, HG, HD)))
                        # the transpose is deferred into the filler stream so
                        # the SP queue never parks on its norm dependency
                        filler.append(("d", emit_transpose, (j, hp, o_sb)))
                filler.extend(
                    ("d", emit_outproj_group, (j, tt)) for tt in range(4))
            # tail flush: scores/AV psum is free now, so let out-proj groups
            # pipeline through the scores pool instead of the single fpool slot
            while filler:
                kind, fn, args = filler.popleft()
                if fn is emit_outproj_group:
                    fn(*args, pool=spool)
                else:
                    fn(*args)

        if iters is None:
            emit_body()
        else:
            with tc.For_i(0, iters, 1):
                emit_body()
    nc.compile()
    return nc


def _get_program():
    global _PROG
    if _PROG is None:
        _PROG = _build_program()
    return _PROG


def run(inputs, trace=False):
    x = np.asarray(inputs["x"], np.float32)
    y = np.asarray(inputs["y"], np.float32)
    Wq = np.asarray(inputs["Wq"], np.float32)
    Wk = np.asarray(inputs["Wk"], np.float32)
    Wv = np.asarray(inputs["Wv"], np.float32)
    Wo = np.asarray(inputs["Wo"], np.float32)
    bq = np.asarray(inputs["bq"], np.float32)
    bk = np.asarray(inputs["bk"], np.float32)
    bv = np.asarray(inputs["bv"], np.float32)
    bo = np.asarray(inputs["bo"], np.float32)

    nc = _get_program()
    tri = (np.arange(P)[None, :] >= np.arange(P)[:, None]).astype(
        np.float16).reshape(P, 1, P)
    ones_c = np.ones((P, HG), np.float16)
    xTs = [np.ascontiguousarray(x[b].T.astype(np.float16)) for b in range(B)]
    yTs = [np.ascontiguousarray(y[b].T.astype(np.float16)) for b in range(B)]

    in_maps = []
    for c in range(8):
        b, hg = divmod(c, HG)
        sl = slice(DL * hg, DL * hg + DL)
        in_maps.append({
            "xT": xTs[b],
            "yT": yTs[b],
            "wq": np.ascontiguousarray(Wq[:, sl].astype(np.float16)),
            "wk": np.ascontiguousarray(Wk[:, sl].astype(np.float16)),
            "wv": np.ascontiguousarray(Wv[:, sl].astype(np.float16)),
            "wo": np.ascontiguousarray(Wo[sl, :].astype(np.float16)),
            "bq": np.ascontiguousarray(bq[sl]),
            "bk": np.ascontiguousarray(bk[sl]),
            "tri": tri,
            "ones_c": ones_c,
        })

    res = run_bass_kernel_spmd(nc, in_maps, list(range(8)), trace=trace)
    extra = bv @ Wo + bo
    out = np.empty((B, S, D), np.float32)
    for b in range(B):
        acc = res.results[HG * b]["out"].astype(np.float32)
        for hg in range(1, HG):
            acc = acc + res.results[HG * b + hg]["out"].astype(np.float32)
        out[b] = acc + extra
    return out, res


def kernel(**inputs):
    out, _ = run(inputs, trace=False)
    return out
